# revision 10
# baseline (speedup 1.0000x reference)
"""APPNP node classifier on 8 TRN2 NeuronCores.

Strategy (graph/data parallel, per sharding hint):
  - Nodes sharded across 8 cores (6250 nodes/core); MLP weights replicated.
  - MLP (x @ W1.T -> relu -> @ W2.T) computed feature-major (zT = [64, nodes]).
  - 10 APPNP propagation steps. Each step:
      * AllGather the per-core h rows -> full h [50000, 64] in HBM (per core copy)
      * dma_gather (SWDGE) fetches h[src] rows (256B each) for this core's edges,
        in a host-precomputed chunk order (128 edges/chunk)
      * per chunk, one PE matmul with a host-built scatter matrix S [128, 16]
        (edge weight at the edge's destination column) accumulates the
        segment sum into PSUM agg tiles [64, 128] (feature-major)
      * alpha mix hnew = 0.9*agg + 0.1*z fused on DVE, PE-transpose back to
        row-major, DMA rows out.
  - Self-loops handled as extra edges with weight deg^-1/2 * deg^-1/2.
  - int16 gather indices only reach 32767, so edges are split into two gather
    calls per group: src < 32768 (base row 0) and src >= 32768 (base row 32768).

The chunk schedule (window starts, chunk counts) is baked into the single SPMD
program, computed as a max over all 8 cores; each core's S / index data pads
its unused chunk slots with index 0 and zero weights.
"""

import os
import sys
import types

for _p in ("/opt/trn_rl_repo", "/root/.axon_site/_ro/trn_rl_repo", "/root/.axon_site"):
    if os.path.isdir(_p) and _p not in sys.path:
        sys.path.insert(0, _p)

import numpy as np

# ---------------------------------------------------------------- config

FULL_CFG = dict(
    N=50000,
    IN_C=512,
    HID_C=256,
    OUT_C=64,
    K=10,
    ALPHA=0.1,
    HALF=32768,
    WMAX=16,
    GT=4,  # dst-tiles per gather group
    MLP_BLK=512,
)

NCORES = 8
P = 128


# ---------------------------------------------------------------- host preprocessing


def _schedule_and_tensors(edge_index, cfg):
    """Build the baked chunk schedule and per-core S / index tensors."""
    N = cfg["N"]
    NPC = N // NCORES
    HALF = cfg["HALF"]
    WMAX = cfg["WMAX"]
    TILES = (NPC + P - 1) // P

    src = np.asarray(edge_index[0], dtype=np.int64)
    dst = np.asarray(edge_index[1], dtype=np.int64)

    deg = np.bincount(dst, minlength=N).astype(np.float64) + 1.0
    dinv = 1.0 / np.sqrt(deg)
    w_e = (dinv[src] * dinv[dst]).astype(np.float32)

    ar = np.arange(N, dtype=np.int64)
    all_src = np.concatenate([src, ar])
    all_dst = np.concatenate([dst, ar])
    all_w = np.concatenate([w_e, (dinv * dinv).astype(np.float32)])

    core = all_dst // NPC
    ld = all_dst % NPC
    tile_id = ld // P
    col = ld % P
    half = (all_src >= HALF).astype(np.int64)

    key = ((core * TILES + tile_id) * 2 + half) * P + col
    counts = np.bincount(key, minlength=NCORES * TILES * 2 * P).reshape(
        NCORES, TILES, 2, P
    )

    # greedy windows per (tile, half): whole-dst columns, cap 128 edges for the
    # worst core, window width <= WMAX
    schedule = {}
    for t in range(TILES):
        for h in (0, 1):
            c = counts[:, t, h, :]  # [8, P]
            csum = np.cumsum(c, axis=1)
            chunks_th = []
            s = 0
            while s < P:
                if c[:, s].max() == 0:
                    s += 1
                    continue
                base = csum[:, s - 1] if s > 0 else np.zeros(NCORES, np.int64)
                e = s
                while e < min(s + WMAX, P) and (csum[:, e] - base).max() <= P:
                    e += 1
                assert e > s, f"dst column with >128 edges at tile {t}"
                chunks_th.append((s, e))
                s = e
            schedule[(t, h)] = chunks_th

    GT = cfg["GT"]
    NGROUPS = (TILES + GT - 1) // GT
    chunk_order = []  # (t, h, s, e)
    groups = []  # per group: dict(start, nA, nB, tiles)
    for g in range(NGROUPS):
        tiles_g = list(range(g * GT, min((g + 1) * GT, TILES)))
        a = [(t, 0, s, e) for t in tiles_g for (s, e) in schedule[(t, 0)]]
        b = [(t, 1, s, e) for t in tiles_g for (s, e) in schedule[(t, 1)]]
        groups.append(dict(start=len(chunk_order), nA=len(a), nB=len(b), tiles=tiles_g))
        chunk_order += a + b
    NCHUNK = len(chunk_order)

    # map (tile, half, col) -> global chunk id + window start
    chunk_of = np.full((TILES, 2, P), -1, np.int64)
    s_of = np.zeros(NCHUNK, np.int64)
    for cg, (t, h, s, e) in enumerate(chunk_order):
        chunk_of[t, h, s:e] = cg
        s_of[cg] = s

    cg_e = chunk_of[tile_id, half, col]
    assert (cg_e >= 0).all()

    # slot within (core, chunk)
    okey = core * NCHUNK + cg_e
    order = np.argsort(okey, kind="stable")
    sk = okey[order]
    is_start = np.ones(len(sk), bool)
    is_start[1:] = sk[1:] != sk[:-1]
    grp_start = np.maximum.accumulate(np.where(is_start, np.arange(len(sk)), 0))
    slot = np.arange(len(sk)) - grp_start
    assert slot.max() < P

    e_core = core[order]
    e_cg = cg_e[order]
    e_src = all_src[order]
    e_w = all_w[order]
    e_col = col[order]
    e_half = half[order]

    IDX = np.zeros((NCORES, NCHUNK, P), np.int64)
    S = np.zeros((NCORES, NCHUNK, P, WMAX), np.float32)
    IDX[e_core, e_cg, slot] = e_src - HALF * e_half
    S[e_core, e_cg, slot, e_col - s_of[e_cg]] = e_w
    assert IDX.max() < 32768 and IDX.min() >= 0

    # wrap indices: slot i of chunk -> partition i%16, col chunk*8 + i//16
    IDXw = (
        IDX.astype(np.int16)
        .reshape(NCORES, NCHUNK, P // 16, 16)
        .transpose(0, 3, 1, 2)
        .reshape(NCORES, 16, NCHUNK * (P // 16))
    )
    IDXw = np.tile(IDXw, (1, 8, 1))  # replicate across the 8 gpsimd cores

    Sw = S.transpose(0, 2, 1, 3).reshape(NCORES, P, NCHUNK * WMAX)

    meta = dict(
        NPC=NPC,
        TILES=TILES,
        NCHUNK=NCHUNK,
        NGROUPS=NGROUPS,
        groups=groups,
        chunk_order=chunk_order,
        WMAX=WMAX,
    )
    return meta, Sw, IDXw


# ---------------------------------------------------------------- device program


def _build_program(cfg, meta):
    from concourse import bass, bacc, mybir
    import concourse.tile as tile
    from concourse.masks import make_identity

    f32 = mybir.dt.float32
    i16 = mybir.dt.int16

    N = cfg["N"]
    IN_C = cfg["IN_C"]
    HID_C = cfg["HID_C"]
    OUT_C = cfg["OUT_C"]
    K = cfg["K"]
    ALPHA = cfg["ALPHA"]
    HALF = cfg["HALF"]
    WMAX = cfg["WMAX"]
    NPC = meta["NPC"]
    TILES = meta["TILES"]
    NCHUNK = meta["NCHUNK"]
    groups = meta["groups"]
    chunk_order = meta["chunk_order"]
    MLP_BLK = cfg["MLP_BLK"]
    ICH = IN_C // P  # input-feature chunks of 128
    OH = HID_C // P  # hidden halves of 128

    nc = bacc.Bacc(
        "TRN2",
        target_bir_lowering=False,
        debug=False,
        num_devices=NCORES,
        num_swdge_queues=4,
    )

    xT_d = nc.dram_tensor("xT", [IN_C, NPC], f32, kind="ExternalInput")
    W1T_d = nc.dram_tensor("W1T", [IN_C, HID_C], f32, kind="ExternalInput")
    W2T_d = nc.dram_tensor("W2T", [HID_C, OUT_C], f32, kind="ExternalInput")
    b1_d = nc.dram_tensor("b1c", [HID_C, 1], f32, kind="ExternalInput")
    b2_d = nc.dram_tensor("b2c", [OUT_C, 1], f32, kind="ExternalInput")
    S_d = nc.dram_tensor("Sw", [P, NCHUNK * WMAX], f32, kind="ExternalInput")
    IDX_d = nc.dram_tensor("IDXw", [P, NCHUNK * 8], i16, kind="ExternalInput")
    out_d = nc.dram_tensor("out", [NPC, OUT_C], f32, kind="ExternalOutput")

    hbuf = [
        nc.dram_tensor(f"hfull{i}", [N, OUT_C], f32, kind="Internal")
        for i in range(2)
    ]
    rows_b = nc.dram_tensor("rows_b", [NPC, OUT_C], f32, kind="Internal")

    with tile.TileContext(nc) as tc:
        with tc.tile_pool(name="persist", bufs=1) as pp:
            S_sb = pp.tile([P, NCHUNK, WMAX], f32)
            nc.sync.dma_start(S_sb[:].rearrange("p a b -> p (a b)"), S_d[:, :])
            idx_sb = pp.tile([P, NCHUNK * 8], i16)
            nc.sync.dma_start(idx_sb[:], IDX_d[:, :])
            W1T_sb = pp.tile([P, ICH, HID_C], f32)
            for i in range(ICH):
                nc.sync.dma_start(W1T_sb[:, i, :], W1T_d[P * i : P * (i + 1), :])
            W2T_sb = pp.tile([P, OH, OUT_C], f32)
            for i in range(OH):
                nc.sync.dma_start(W2T_sb[:, i, :], W2T_d[P * i : P * (i + 1), :])
            b1_sb = pp.tile([P, OH], f32)
            for i in range(OH):
                nc.sync.dma_start(b1_sb[:, i : i + 1], b1_d[P * i : P * (i + 1), :])
            b2_sb = pp.tile([OUT_C, 1], f32)
            nc.sync.dma_start(b2_sb[:], b2_d[:, :])
            b2s_sb = pp.tile([OUT_C, 1], f32)
            nc.scalar.mul(b2s_sb[:], b2_sb[:], ALPHA)
            ident = pp.tile([OUT_C, OUT_C], f32)
            make_identity(nc, ident[:])
            zTs = pp.tile([OUT_C, TILES * P], f32)  # ALPHA * z, feature-major
            nc.vector.memset(zTs[:], 0.0)

            # ---------------- MLP ----------------
            nblk = (NPC + MLP_BLK - 1) // MLP_BLK
            with tc.tile_pool(name="mlp_sb", bufs=2) as mp, tc.tile_pool(
                name="mlp_ps", bufs=2, space="PSUM"
            ) as mps, tc.tile_pool(name="mlp_ps2", bufs=2, space="PSUM") as mps2, tc.tile_pool(
                name="mlp_tr", bufs=2, space="PSUM"
            ) as mtr:
                for b in range(nblk):
                    c0 = b * MLP_BLK
                    W = min(MLP_BLK, NPC - c0)
                    xt = mp.tile([P, ICH, MLP_BLK], f32, tag="xt")
                    for i in range(ICH):
                        nc.sync.dma_start(
                            xt[:, i, :W], xT_d[P * i : P * (i + 1), c0 : c0 + W]
                        )
                    h1 = mp.tile([P, OH, MLP_BLK], f32, tag="h1")
                    for o in range(OH):
                        ps = mps.tile([P, MLP_BLK], f32, tag="psh")
                        for i in range(ICH):
                            nc.tensor.matmul(
                                ps[:, :W],
                                lhsT=W1T_sb[:, i, P * o : P * (o + 1)],
                                rhs=xt[:, i, :W],
                                start=(i == 0),
                                stop=(i == ICH - 1),
                            )
                        nc.scalar.activation(
                            h1[:, o, :W],
                            ps[:, :W],
                            mybir.ActivationFunctionType.Relu,
                            bias=b1_sb[:, o : o + 1],
                        )
                    psz = mps2.tile([OUT_C, MLP_BLK], f32, tag="psz")
                    for o in range(OH):
                        nc.tensor.matmul(
                            psz[:, :W],
                            lhsT=W2T_sb[:, o, :],
                            rhs=h1[:, o, :W],
                            start=(o == 0),
                            stop=(o == OH - 1),
                        )
                    # scaled copy for the mix, and plain rows for h0
                    nc.scalar.activation(
                        zTs[:, c0 : c0 + W],
                        psz[:, :W],
                        mybir.ActivationFunctionType.Identity,
                        bias=b2s_sb[:],
                        scale=ALPHA,
                    )
                    zp = mp.tile([OUT_C, MLP_BLK], f32, tag="zp")
                    nc.scalar.activation(
                        zp[:, :W],
                        psz[:, :W],
                        mybir.ActivationFunctionType.Identity,
                        bias=b2_sb[:],
                    )
                    rows_t0 = out_d if cfg.get("MLP_ONLY") else rows_b
                    for j in range((W + P - 1) // P):
                        r = min(P, W - j * P)
                        ptr = mtr.tile([P, OUT_C], f32, tag="ptr")
                        nc.tensor.transpose(
                            ptr[:r, :], zp[:, j * P : j * P + r], ident[:]
                        )
                        zr = mp.tile([P, OUT_C], f32, tag="zr")
                        nc.scalar.copy(zr[:r, :], ptr[:r, :])
                        nc.sync.dma_start(
                            rows_t0[c0 + j * P : c0 + j * P + r, :], zr[:r, :]
                        )
            if cfg.get("MLP_ONLY"):
                pass
            else:
                _prop(
                    nc, tc, cfg, meta, hbuf, rows_b, out_d, idx_sb, S_sb, zTs, ident
                )

    nc.compile()
    return nc


def _prop(nc, tc, cfg, meta, hbuf, rows_b, out_d, idx_sb, S_sb, zTs, ident):
    from concourse import mybir
    import concourse.tile as tile

    f32 = mybir.dt.float32
    N = cfg["N"]
    OUT_C = cfg["OUT_C"]
    K = cfg["K"]
    ALPHA = cfg["ALPHA"]
    HALF = cfg["HALF"]
    WMAX = cfg["WMAX"]
    NPC = meta["NPC"]
    groups = meta["groups"]
    chunk_order = meta["chunk_order"]
    NCORES = 8

    if True:
            nc.gpsimd.collective_compute(
                "AllGather",
                mybir.AluOpType.bypass,
                replica_groups=[list(range(NCORES))],
                ins=[rows_b[:, :].opt()],
                outs=[hbuf[0][:, :].opt()],
            )

            # ---------------- propagation ----------------
            with tc.tile_pool(name="prop_sb", bufs=8) as gp, tc.tile_pool(
                name="prop_sb2", bufs=3
            ) as hp, tc.tile_pool(name="prop_ps", bufs=4, space="PSUM") as aps, tc.tile_pool(
                name="prop_tr", bufs=2, space="PSUM"
            ) as tps:
                for step in range(K):
                    h_src = hbuf[step % 2]
                    last = step == K - 1
                    rows_target = out_d if last else rows_b
                    qrr = 0
                    for g in groups:
                        nA, nB = g["nA"], g["nB"]
                        c0 = g["start"]
                        ng = nA + nB
                        # SWDGE ring holds 1024 descriptors -> max 8 chunks
                        # (1024 rows) per dma_gather call; round-robin the 4
                        # queues so the Q7 DSP pairs generate in parallel
                        slabs = {}
                        for lo, hi, src_ap in (
                            (c0, c0 + nA, h_src[0:HALF, :]),
                            (c0 + nA, c0 + ng, h_src[HALF:N, :]),
                        ):
                            cc = lo
                            while cc < hi:
                                n = min(8, hi - cc)
                                G = gp.tile([P, 8, OUT_C], f32, tag="G")
                                nc.gpsimd.dma_gather(
                                    G[:, :n, :],
                                    src_ap,
                                    idx_sb[:, 8 * cc : 8 * (cc + n)],
                                    num_idxs=P * n,
                                    num_idxs_reg=P * n,
                                    elem_size=OUT_C,
                                    queue_num=qrr % 4,
                                )
                                qrr += 1
                                for j in range(n):
                                    slabs[cc + j] = (G, j)
                                cc += n
                        # per tile in this group: chunks are ordered A-tiles then B-tiles
                        tile_chunks = {t: [] for t in g["tiles"]}
                        for lc in range(ng):
                            t, h, s, e = chunk_order[c0 + lc]
                            tile_chunks[t].append((lc, s))
                        for t in g["tiles"]:
                            agg = aps.tile([OUT_C, P + WMAX], f32, tag="agg")
                            nc.vector.memset(agg[:], 0.0)
                            for lc, s in tile_chunks[t]:
                                Gt, j = slabs[c0 + lc]
                                nc.tensor.matmul(
                                    agg[:, s : s + WMAX],
                                    lhsT=Gt[:, j, :],
                                    rhs=S_sb[:, c0 + lc, :],
                                    start=False,
                                    stop=True,
                                    skip_group_check=True,
                                )
                            ht = hp.tile([OUT_C, P], f32, tag="ht")
                            nc.vector.scalar_tensor_tensor(
                                ht[:],
                                agg[:, :P],
                                1.0 - ALPHA,
                                zTs[:, t * P : (t + 1) * P],
                                op0=mybir.AluOpType.mult,
                                op1=mybir.AluOpType.add,
                            )
                            r = min(P, NPC - t * P)
                            ptr = tps.tile([P, OUT_C], f32, tag="ptr2")
                            nc.tensor.transpose(ptr[:r, :], ht[:, :r], ident[:])
                            hr = hp.tile([P, OUT_C], f32, tag="hr")
                            nc.scalar.copy(hr[:r, :], ptr[:r, :])
                            nc.sync.dma_start(
                                rows_target[t * P : t * P + r, :], hr[:r, :]
                            )
                    if not last:
                        nc.gpsimd.collective_compute(
                            "AllGather",
                            mybir.AluOpType.bypass,
                            replica_groups=[list(range(NCORES))],
                            ins=[rows_b[:, :].opt()],
                            outs=[hbuf[(step + 1) % 2][:, :].opt()],
                        )


# ---------------------------------------------------------------- runner

_CACHE = {}


def _get_program(edge_index, cfg):
    key = ("prog", cfg["N"], int(np.asarray(edge_index).sum() & 0xFFFFFFFF))
    if key not in _CACHE:
        meta, Sw, IDXw = _schedule_and_tensors(edge_index, cfg)
        nc = _build_program(cfg, meta)
        _CACHE[key] = (nc, meta, Sw, IDXw)
    return _CACHE[key]


def kernel(x, edge_index, W1, b1, W2, b2, _cfg=None):
    cfg = dict(FULL_CFG if _cfg is None else _cfg)
    x = np.ascontiguousarray(np.asarray(x, dtype=np.float32))
    edge_index = np.asarray(edge_index)
    W1 = np.asarray(W1, dtype=np.float32)
    b1 = np.asarray(b1, dtype=np.float32)
    W2 = np.asarray(W2, dtype=np.float32)
    b2 = np.asarray(b2, dtype=np.float32)

    nc, meta, Sw, IDXw = _get_program(edge_index, cfg)
    NPC = meta["NPC"]

    W1T = np.ascontiguousarray(W1.T)
    W2T = np.ascontiguousarray(W2.T)
    b1c = np.ascontiguousarray(b1.reshape(-1, 1))
    b2c = np.ascontiguousarray(b2.reshape(-1, 1))

    in_maps = []
    for c in range(NCORES):
        xT_c = np.ascontiguousarray(x[c * NPC : (c + 1) * NPC].T)
        in_maps.append(
            dict(
                xT=xT_c,
                W1T=W1T,
                W2T=W2T,
                b1c=b1c,
                b2c=b2c,
                Sw=np.ascontiguousarray(Sw[c]),
                IDXw=np.ascontiguousarray(IDXw[c]),
            )
        )

    from concourse import bass_utils

    res = bass_utils.run_bass_kernel_spmd(
        nc, in_maps, core_ids=list(range(NCORES)), trace=bool(os.environ.get("APPNP_TRACE"))
    )
    out = np.concatenate([res.results[c]["out"] for c in range(NCORES)], axis=0)
    kernel.last_exec_time_ns = res.exec_time_ns
    kernel.last_results = res
    return out
